# revision 10
# baseline (speedup 1.0000x reference)
"""Trainium2 Bass kernel for fused LoRA-attention block (nn_Attention_18846316494887).

Reference computation:
  qkv = y @ Wqkv.T + bqkv (+ LoRA deltas y @ (B@A) per Q/K/V)  -> Q,K,V [B,H,S,D]
  attn = softmax(Q K^T / sqrt(D)); o = attn @ V -> [B,S,E]
  msa = o @ Wmsa.T + o @ (Bo@Ao); res = msa + y; out = LayerNorm(res)*gamma + beta

Sharding: tensor-parallel over heads (2 heads/core, 8 cores), AllToAll to
reshard head-dim -> token-dim before the output projection, token-parallel
msa + LayerNorm, host-side gather of per-core token shards.

Precision plan (error budget: attention path contributes only ~2.2% of the
LN'd output norm, so a few-% relative error there is invisible):
  - y, Wqkv (x32), V, exp(scores) all in fp8e4m3; f32 PSUM accumulation
  - Q/K projection matmuls in DoubleRow mode (2 fp8 k-subtiles per pass)
  - AV matmuls in DoubleRow mode over kt-pairs (halves the ex stream time)
  - the x32*x32 weight scaling and 1/sqrt(D) fold into the exp's free
    affine scale (exp(x * 1/8192)); V-scale folds into msa weights (/32)

Host-side prep (exact algebra, no approximation):
  - LoRA folded into Wqkv / Wmsa (y@W.T + y@(B@A) == y@(W.T + B@A))
  - V bias folded into the residual shard: (o+bv)@M + y == o@M + (y + bv@M)
  - y pre-transposed to [E, T] for the QKV matmuls

Schedule (v1): startup DMAs striped over 4 engine queues (first exp ~4us);
softmax finalize normalizes straight from PSUM into the A2A DRAM payload
(recip at psum partition 0 via ones-first V padding); msa+LayerNorm for
shards 0-2 interleave into b1's attention as background PE/DVE work (LN
rstd via poly+Newton on DVE, so the ACT exp table is never swapped);
warm-up dummies keep the PE p-state high over the last collective.
"""
import functools
import numpy as np
import ml_dtypes

import concourse.mybir as mybir
import concourse.tile as tile
from concourse import bacc
from concourse import bass_utils

# problem shapes (hardcoded per harness contract)
E = 1024
H = 16
D = 64
B = 2
S = 2048
T = B * S          # 4096 tokens
N_CORES = 8
EPS = 1e-6

BF16 = mybir.dt.bfloat16
F32 = mybir.dt.float32
F8 = mybir.dt.float8e4
NP_F8 = ml_dtypes.float8_e4m3
AF = mybir.ActivationFunctionType
ALU = mybir.AluOpType
DR = mybir.MatmulPerfMode.DoubleRow

# per-core worksizes
TOK = T // N_CORES          # 512 tokens per core for msa/LN
QC = 512                    # attention q-chunk
N_QC = S // QC              # 4 q-chunks per (b, head-pair)
N_KT = S // 128             # 16 k-tiles
N_KP = N_KT // 2            # 8 kt-pairs (DoubleRow AV granularity)
VW = 80                     # padded V row (1 ones + 64 d + pad to 16B mult)
WSC = 32.0                  # fp8 weight pre-scale
S_ACT = 1.0 / (WSC * WSC * 8.0)   # exp affine scale: /32^2 (w-scales) /sqrt(D)

# quadratic seed for 1/sqrt(v) on v in [0.4, 3.0]; 3 Newton steps follow
RSQ_C2 = 0.131712920391688
RSQ_C1 = -0.7403870149754509
RSQ_C0 = 1.6463305621254531


def _build(use_gamma: bool, use_beta: bool):
    nc = bacc.Bacc("TRN2", target_bir_lowering=False, debug=False, num_devices=N_CORES)

    # ---- DRAM parameters -------------------------------------------------
    yT = nc.dram_tensor("yT", [E, T], F8, kind="ExternalInput")
    wqT = nc.dram_tensor("wqT", [E, 128], F8, kind="ExternalInput")
    wkT = nc.dram_tensor("wkT", [E, 128], F8, kind="ExternalInput")
    wvT = nc.dram_tensor("wvT", [E, 128], F8, kind="ExternalInput")
    bq = nc.dram_tensor("bq", [128, 1], F32, kind="ExternalInput")
    bk = nc.dram_tensor("bk", [128, 1], F32, kind="ExternalInput")
    msa_w = nc.dram_tensor("msa_w", [E, E], F8, kind="ExternalInput")
    y_shard = nc.dram_tensor("y_shard", [TOK, E], F32, kind="ExternalInput")
    if use_gamma:
        gamma_b = nc.dram_tensor("gamma_b", [128, E], F32, kind="ExternalInput")
    if use_beta:
        beta_b = nc.dram_tensor("beta_b", [128, E], F32, kind="ExternalInput")
    out = nc.dram_tensor("out", [TOK, E], F32, kind="ExternalOutput")

    # internal DRAM: A2A bounce buffers (shard k: (b, q-half) -> 128 tok/core)
    a2a_in = [nc.dram_tensor(f"a2a_in{k}", [N_CORES, 128, 128], F8) for k in range(4)]
    a2a_out = [nc.dram_tensor(f"a2a_out{k}", [N_CORES, 128, 128], F8) for k in range(4)]

    with tile.TileContext(nc) as tc:
        with (
            tc.tile_pool(name="const", bufs=1) as cpool,
            tc.tile_pool(name="yt", bufs=5) as ytp,
            tc.tile_pool(name="qk", bufs=1) as qkp,
            tc.tile_pool(name="exp", bufs=3) as expp,
            tc.tile_pool(name="stage", bufs=1) as stp,
            tc.tile_pool(name="fin", bufs=2) as finp,
            tc.tile_pool(name="a2asb", bufs=2) as a2ap,
            tc.tile_pool(name="ps_acc", bufs=2, space="PSUM") as ps_acc,
            tc.tile_pool(name="ps_sc", bufs=2, space="PSUM") as ps_sc,
            tc.tile_pool(name="ps_av", bufs=2, space="PSUM") as ps_av,
        ):
            # ---- constants + all bulk DMAs, striped over 4 engine queues ----
            wqT_sb = cpool.tile([128, 8, 128], F8)
            wkT_sb = cpool.tile([128, 8, 128], F8)
            wvT_sb = cpool.tile([128, 8, 128], F8)
            nc.sync.dma_start(wqT_sb[:], wqT[:, :].rearrange("(a p) n -> p a n", p=128))
            nc.gpsimd.dma_start(wkT_sb[:], wkT[:, :].rearrange("(a p) n -> p a n", p=128))
            nc.scalar.dma_start(wvT_sb[:], wvT[:, :].rearrange("(a p) n -> p a n", p=128))
            bq_sb = cpool.tile([128, 1], F32)
            bk_sb = cpool.tile([128, 1], F32)
            nc.scalar.dma_start(bq_sb[:], bq[:, :])
            nc.scalar.dma_start(bk_sb[:], bk[:, :])
            msa_w_sb = cpool.tile([128, 8, E], F8)
            y_shard_sb = cpool.tile([128, 4, E], F32)
            if use_gamma:
                gamma_sb = cpool.tile([128, E], F32)
            if use_beta:
                beta_sb = cpool.tile([128, E], F32)

            # y tiles: b0 chunk0 striped over all 4 queues (fastest first
            # scores), b0 chunks 1-3 over sync+gpsimd, b1 all on sync (the
            # scalar/vector queues must be clear once ACT/DVE work starts).
            yts = {0: [], 1: []}
            q3 = [nc.sync, nc.gpsimd, nc.scalar]
            for b in (0, 1):
                for tc8 in range(4):
                    yt = ytp.tile([128, 8, 512], F8, tag="yt")
                    yts[b].append(yt)
                    for et in range(8):
                        if b == 1:
                            eng = nc.sync
                        elif tc8 == 0:
                            eng = q3[et % 3]
                        else:
                            eng = nc.sync if et % 2 == 0 else nc.gpsimd
                        eng.dma_start(
                            yt[:, et, :], yT[128 * et:128 * (et + 1),
                                             b * S + 512 * tc8: b * S + 512 * (tc8 + 1)])
            # warm the gpsimd broadcast library now (LOAD_LIB stalls the Q7s
            # ~9us one-time); placed after the gp load DMAs so it runs in the
            # attention window where nothing waits on gpsimd yet
            libw = cpool.tile([1, 16], F32, name="libw")
            libw2 = cpool.tile([2, 16], F32, name="libw2")
            nc.vector.memset(libw[0:1, :], 1.0)
            nc.gpsimd.partition_broadcast(libw2[:, :], libw[0:1, :])
            # bulk consts after the y tiles on sync: ready well before the
            # msa/LN interleave needs them (~80us) without delaying b1 loads
            nc.sync.dma_start(msa_w_sb[:], msa_w[:, :].rearrange("(a p) n -> p a n", p=128))
            nc.sync.dma_start(y_shard_sb[:], y_shard[:, :].rearrange("(a p) n -> p a n", p=128))
            if use_gamma:
                nc.sync.dma_start(gamma_sb[:], gamma_b[:, :])
            if use_beta:
                nc.sync.dma_start(beta_sb[:], beta_b[:, :])

            # V tiles, padded: [k-part, b, head, kt, VW]; col 64 = ones (the
            # softmax denominator lands on PSUM partition 64 -- 32-aligned
            # PSUM access keeps the DVE verifier happy)
            v_sb = cpool.tile([128, B, 2, N_KT, VW], F8)
            nc.vector.memset(v_sb[:, :, :, :, 64:VW], 0.0)
            nc.vector.memset(v_sb[:, :, :, :, 64:65], 1.0)

            # Q^T/K^T: [d-part(2 heads), b, q]
            qT_sb = qkp.tile([128, B, S], BF16)
            kT_sb = qkp.tile([128, B, S], BF16)
            # residual rows for the 4 token shards
            res_sb = stp.tile([128, 4, E], F32)
            mu_t = [cpool.tile([128, 6], F32, name=f"mu{k}") for k in range(4)]

            # ============== QKV projection step factories ==============
            def make_qkv_steps(b):
                """QKV projection for batch b as small closures so the PE work
                interleaves into attention (fills ACT-wait slots)."""
                qs, ks, vs = [], [], []
                for tc8 in range(4):
                    st8 = {"yt": yts[b][tc8]}
                    qs.append([])
                    ks.append([])
                    vs.append([])

                    # Q/K: 4 DoubleRow matmuls (et-pairs), K=1024 contraction.
                    for eg in range(4):
                        def qstep(b=b, tc8=tc8, eg=eg, st8=st8):
                            if eg == 0:
                                st8["ps_q"] = ps_acc.tile([128, 512], F32, tag="acc", name="ps_q")
                            ps_q, yt = st8["ps_q"], st8["yt"]
                            st, sp = (eg == 0), (eg == 3)
                            nc.tensor.matmul(ps_q[:], wqT_sb[:, 2 * eg:2 * eg + 2, :],
                                             yt[:, 2 * eg:2 * eg + 2, :], start=st, stop=sp,
                                             perf_mode=DR)
                            if eg == 3:
                                nc.vector.tensor_scalar(
                                    qT_sb[:, b, 512 * tc8:512 * (tc8 + 1)], ps_q[:],
                                    bq_sb[:], None, ALU.add)
                        qs[tc8].append(qstep)

                    for eg in range(4):
                        def kstep(b=b, tc8=tc8, eg=eg, st8=st8):
                            if eg == 0:
                                st8["ps_k"] = ps_acc.tile([128, 512], F32, tag="acc", name="ps_k")
                            ps_k, yt = st8["ps_k"], st8["yt"]
                            st, sp = (eg == 0), (eg == 3)
                            nc.tensor.matmul(ps_k[:], wkT_sb[:, 2 * eg:2 * eg + 2, :],
                                             yt[:, 2 * eg:2 * eg + 2, :], start=st, stop=sp,
                                             perf_mode=DR)
                            if eg == 3:
                                nc.vector.tensor_scalar(
                                    kT_sb[:, b, 512 * tc8:512 * (tc8 + 1)], ps_k[:],
                                    bk_sb[:], None, ALU.add)
                        ks[tc8].append(kstep)

                    # V: [tok, vdim] layout, fp8 operands (no DoubleRow: the
                    # stationary operand changes every matmul, FWL covers it)
                    for eg in range(4):
                        def vstep(b=b, tc8=tc8, eg=eg, st8=st8):
                            if eg == 0:
                                st8["ps_v"] = ps_acc.tile([128, 512], F32, tag="acc", name="ps_v")
                            ps_v, yt = st8["ps_v"], st8["yt"]
                            for et in (2 * eg, 2 * eg + 1):
                                st, sp = (et == 0), (et == 7)
                                for s4 in range(4):
                                    nc.tensor.matmul(ps_v[:, 128 * s4:128 * (s4 + 1)],
                                                     yt[:, et, 128 * s4:128 * (s4 + 1)],
                                                     wvT_sb[:, et, :], start=st, stop=sp)
                            if eg == 3:
                                for h in range(2):
                                    src = ps_v[:, :].rearrange(
                                        "p (s n) -> p s n", s=4)[:, :, 64 * h:64 * (h + 1)]
                                    nc.vector.tensor_copy(
                                        v_sb[:, b, h, 4 * tc8:4 * (tc8 + 1), 0:64], src)
                        vs[tc8].append(vstep)
                return qs, ks, vs

            # ============== attention ==============
            def attention(b, bg, av_last=None, qcs=range(N_QC), pops=2):
                # software-pipelined ACROSS kt steps: qk/exp runs OV steps
                # ahead of av; bg closures fill the PE's ACT-wait slots.
                if av_last is None:
                    av_last = []
                OV = 4
                states = {}

                def qk_exp(qc, kt):
                    stq = states[qc]
                    if kt % 2 == 0:
                        stq["exs"][kt // 2] = expp.tile([128, 2, 1024], F8, name="ex")
                    sc = ps_sc.tile([128, 1024], F32, tag="sc", name="sc")
                    nc.tensor.matmul(sc[:, 0:512],
                                     kT_sb[0:64, b, 128 * kt:128 * (kt + 1)],
                                     qT_sb[0:64, b, QC * qc:QC * (qc + 1)],
                                     start=True, stop=True, tile_position=(0, 0))
                    nc.tensor.matmul(sc[:, 512:1024],
                                     kT_sb[64:128, b, 128 * kt:128 * (kt + 1)],
                                     qT_sb[64:128, b, QC * qc:QC * (qc + 1)],
                                     start=True, stop=True, tile_position=(64, 0))
                    ex = stq["exs"][kt // 2]
                    nc.scalar.activation(ex[:, kt % 2, :], sc[:], AF.Exp, scale=S_ACT)

                def av_a(qc, kp):
                    stq = states[qc]
                    if kp == 0:
                        stq["av_a"] = ps_av.tile([128, 512], F32, tag="av", name="av_a")
                        stq["av_b"] = ps_av.tile([128, 512], F32, tag="av", name="av_b")
                    ex = stq["exs"][kp]
                    nc.tensor.matmul(stq["av_a"][0:65, :],
                                     v_sb[:, b, 0, 2 * kp:2 * kp + 2, 0:65],
                                     ex[:, :, 0:512],
                                     start=(kp == 0), stop=(kp == N_KP - 1), perf_mode=DR)

                def av_b(qc, kp):
                    stq = states[qc]
                    ex = stq["exs"][kp]
                    i2 = nc.tensor.matmul(stq["av_b"][0:65, :],
                                          v_sb[:, b, 1, 2 * kp:2 * kp + 2, 0:65],
                                          ex[:, :, 512:1024],
                                          start=(kp == 0), stop=(kp == N_KP - 1), perf_mode=DR)
                    if kp == N_KP - 1:
                        av_last.append(i2)

                def finalize(qc):
                    # av psum: partitions 0:64 = o rows, partition 64 = denom.
                    # denom -> p0 (DMA hop), recip, broadcast, one fused mult
                    # straight to fp8, DMA into the A2A payload (no staging).
                    av_a, av_b = states[qc]["av_a"], states[qc]["av_b"]
                    den = finp.tile([65, 1024], F32, tag="den", name="den")
                    nc.vector.tensor_copy(den[64:65, 0:512], av_a[64:65, :])
                    nc.vector.tensor_copy(den[64:65, 512:1024], av_b[64:65, :])
                    rc = finp.tile([1, 1024], F32, tag="rc", name="rc")
                    nc.gpsimd.dma_start(rc[0:1, :], den[64:65, :])
                    rc2 = finp.tile([1, 1024], F32, tag="rc2", name="rc2")
                    nc.vector.reciprocal_approx_fast(rc2[0:1, :], rc[0:1, :])
                    rb = finp.tile([64, 1024], F32, tag="rb", name="rb")
                    nc.gpsimd.partition_broadcast(rb[:, :], rc2[0:1, :])
                    ta = finp.tile([64, 512], F8, tag="ta", name="ta")
                    tb = finp.tile([64, 512], F8, tag="tb", name="tb")
                    nc.vector.tensor_tensor(ta[:, :], av_a[0:64, :],
                                            rb[:, 0:512], ALU.mult)
                    nc.vector.tensor_tensor(tb[:, :], av_b[0:64, :],
                                            rb[:, 512:1024], ALU.mult)
                    hf = qc // 2
                    qh = qc % 2
                    k = 2 * b + hf
                    nc.gpsimd.dma_start(
                        a2a_in[k].ap()[4 * qh:4 * qh + 4, 0:64, :].rearrange("j p n -> p j n"),
                        ta[:, :].rearrange("p (j n) -> p j n", j=4))
                    nc.gpsimd.dma_start(
                        a2a_in[k].ap()[4 * qh:4 * qh + 4, 64:128, :].rearrange("j p n -> p j n"),
                        tb[:, :].rearrange("p (j n) -> p j n", j=4))
                    if qh == 1:
                        nc.gpsimd.collective_compute(
                            "AllToAll", ALU.bypass,
                            replica_groups=[list(range(N_CORES))],
                            ins=[a2a_in[k].ap().opt()],
                            outs=[a2a_out[k].ap().opt()],
                        )

                seq = [(qc, kt) for qc in qcs for kt in range(N_KT)]
                for i, (qc, kt) in enumerate(seq):
                    states.setdefault(qc, {"exs": [None] * N_KP})
                    qk_exp(qc, kt)
                    for _ in range(pops):
                        if bg:
                            bg.pop(0)()
                    j = i - OV
                    if j >= 0 and seq[j][1] % 2 == 1:
                        jqc, jkt = seq[j]
                        av_a(jqc, jkt // 2)
                        av_b(jqc, jkt // 2)
                        if jkt == N_KT - 1:
                            finalize(jqc)
                for j in range(max(0, len(seq) - OV), len(seq)):
                    if seq[j][1] % 2 == 1:
                        jqc, jkt = seq[j]
                        av_a(jqc, jkt // 2)
                        av_b(jqc, jkt // 2)
                        if jkt == N_KT - 1:
                            finalize(jqc)
                return av_last

            # ============== msa + residual + LayerNorm step factories ==========
            def msa_ln_steps(k):
                steps = []
                st = {}

                def lhs_load(k=k, st=st):
                    lhs = a2ap.tile([128, 8, 128], F8, tag="lhs")
                    st["lhs"] = lhs
                    nc.sync.dma_start(lhs[:], a2a_out[k].ap().rearrange("j p n -> p j n"))
                steps.append(lhs_load)

                # i-major so consecutive matmuls share lhs weights; both
                # e-halves accumulate concurrently in two psum tiles.
                for i in range(4):
                    for ec in (0, 1):
                        def mm(i=i, ec=ec, st=st):
                            if i == 0 and ec == 0:
                                st["m0"] = ps_acc.tile([128, 512], F32, tag="acc", name="ps_m0")
                                st["m1"] = ps_acc.tile([128, 512], F32, tag="acc", name="ps_m1")
                            ps_m = st["m0"] if ec == 0 else st["m1"]
                            nc.tensor.matmul(ps_m[:], st["lhs"][:, 2 * i:2 * i + 2, :],
                                             msa_w_sb[:, 2 * i:2 * i + 2,
                                                      512 * ec:512 * (ec + 1)],
                                             start=(i == 0), stop=(i == 3),
                                             perf_mode=DR)
                        steps.append(mm)

                for ec in (0, 1):
                    def res_step(ec=ec, k=k, st=st):
                        ps_m = st["m0"] if ec == 0 else st["m1"]
                        nc.vector.tensor_tensor(
                            res_sb[:, k, 512 * ec:512 * (ec + 1)], ps_m[:],
                            y_shard_sb[:, k, 512 * ec:512 * (ec + 1)], ALU.add)
                    steps.append(res_step)

                def ln_a(k=k):
                    stats = finp.tile([128, 2, 6], F32, tag="stats")
                    nc.vector.bn_stats(stats[:, 0, :], res_sb[:, k, 0:512])
                    nc.vector.bn_stats(stats[:, 1, :], res_sb[:, k, 512:1024])
                    mu = mu_t[k]
                    nc.vector.bn_aggr(mu[:, 0:2], stats[:])
                    # res carries a WSC^2=1024 scale, so var carries 1024^2:
                    # v = var/2^20 + eps is the TRUE variance + eps, inside
                    # the rsqrt seed's fit range; negate mean for the apply
                    nc.vector.tensor_scalar(mu[:, 1:2], mu[:, 1:2], 2.0 ** -20, EPS,
                                            ALU.mult, ALU.add)
                    nc.vector.tensor_scalar(mu[:, 0:1], mu[:, 0:1], -1.0, None, ALU.mult)
                steps.append(ln_a)

                def ln_b(k=k):
                    # rstd = 1/sqrt(v) via quadratic seed + 3 Newton steps,
                    # all on DVE (keeps the ACT exp table resident)
                    mu = mu_t[k]
                    v = mu[:, 1:2]
                    y = mu[:, 3:4]
                    t = mu[:, 4:5]
                    nc.vector.tensor_scalar(y, v, RSQ_C2, RSQ_C1, ALU.mult, ALU.add)
                    nc.vector.tensor_tensor(y, y, v, ALU.mult)
                    nc.vector.tensor_scalar(y, y, RSQ_C0, None, ALU.add)
                    for _ in range(3):
                        nc.vector.tensor_tensor(t, y, y, ALU.mult)
                        nc.vector.tensor_tensor(t, t, v, ALU.mult)
                        nc.vector.tensor_scalar(t, t, -0.5, 1.5, ALU.mult, ALU.add)
                        nc.vector.tensor_tensor(y, y, t, ALU.mult)
                    # rstd for the SCALED res rows: 1/sqrt(2^20 * v)
                    nc.vector.tensor_scalar(y, y, 2.0 ** -10, None, ALU.mult)
                steps.append(ln_b)

                def ln_c(k=k):
                    mu = mu_t[k]
                    o1 = finp.tile([128, E], F32, tag="o1")
                    nc.vector.tensor_scalar(o1[:], res_sb[:, k, :], mu[:, 0:1],
                                            mu[:, 3:4], ALU.add, ALU.mult)
                    if use_gamma:
                        nc.vector.tensor_tensor(o1[:], o1[:], gamma_sb[:], ALU.mult)
                    if use_beta:
                        nc.vector.tensor_tensor(o1[:], o1[:], beta_sb[:], ALU.add)
                    nc.sync.dma_start(out[128 * k:128 * (k + 1), :], o1[:])
                steps.append(ln_c)
                return steps

            # ============== drive ==============
            q0, k0, v0 = make_qkv_steps(0)
            for s in k0[0]:
                s()
            for s in q0[0]:
                s()
            for s in v0[0]:
                s()
            q1, k1, v1 = make_qkv_steps(1)
            # bgA feeds b0 qc0: kT chunk c needed by step 4c, v chunk c by the
            # av of its kt-pairs, q0[1] before qc1 starts; 28 closures, 32 slots
            bgA = []
            for tc8 in (1, 2, 3):
                bgA.extend(k0[tc8])
                bgA.extend(v0[tc8])
            bgA.extend(q0[1])
            attention(0, bgA, qcs=[0])
            while bgA:
                bgA.pop(0)()
            # qc1: rest of b0's Q + b1's kT (24 closures, 32 slots)
            bgB = list(q0[2]) + list(q0[3])
            for tc8 in range(4):
                bgB.extend(k1[tc8])
            attention(0, bgB, qcs=[1])
            while bgB:
                bgB.pop(0)()
            # qc2+qc3: b1's Q and V (32 closures, 32 slots)
            bgC = []
            for tc8 in range(4):
                bgC.extend(q1[tc8])
                bgC.extend(v1[tc8])
            attention(0, bgC, qcs=[2, 3], pops=1)
            while bgC:
                bgC.pop(0)()

            # b1 attention with msa+LN for shards 0-2 interleaved as bg work.
            # shard k is ready once collective k lands: k=0,1 during b0, k=2
            # after b1 qc1's finalize -- pad so k=2 starts ~step 44 (mid-qc2,
            # collective 2 done) and never stalls the in-order PE queue.
            noop = lambda: None
            bgD = []
            bgD.extend(msa_ln_steps(0))
            bgD.extend(msa_ln_steps(1))
            bgD.extend([noop] * (44 - len(bgD)))
            bgD.extend(msa_ln_steps(2))
            av_anchors = attention(1, bgD, pops=1)
            while bgD:
                bgD.pop(0)()

            # keep the PE p-state warm across the last A2A's latency
            dmy = ps_sc.tile([128, 1024], F32, tag="sc", name="dmy")
            for i in range(24):
                nc.tensor.matmul(dmy[:, 0:512], wqT_sb[:, 0:2, :],
                                 msa_w_sb[:, 0:2, 0:512], start=True, stop=True,
                                 perf_mode=DR)

            # last shard: msa + LN tail
            for s in msa_ln_steps(3):
                s()

    nc.compile()
    return nc


@functools.lru_cache(maxsize=4)
def _get_nc(use_gamma: bool, use_beta: bool):
    return _build(use_gamma, use_beta)


def kernel(**inputs) -> np.ndarray:
    y = np.asarray(inputs["y"], np.float32)
    Wqkv = np.asarray(inputs["Wqkv"], np.float32)
    bqkv = np.asarray(inputs["bqkv"], np.float32)
    Wmsa = np.asarray(inputs["Wmsa"], np.float32)
    Bq_, Aq_ = np.asarray(inputs["Bq"], np.float32), np.asarray(inputs["Aq"], np.float32)
    Bk_, Ak_ = np.asarray(inputs["Bk"], np.float32), np.asarray(inputs["Ak"], np.float32)
    Bv_, Av_ = np.asarray(inputs["Bv"], np.float32), np.asarray(inputs["Av"], np.float32)
    Bo_, Ao_ = np.asarray(inputs["Bo"], np.float32), np.asarray(inputs["Ao"], np.float32)
    gamma = np.asarray(inputs["gamma"], np.float32)
    beta = np.asarray(inputs["beta"], np.float32)

    # effective weights: qkv = y @ (Wqkv.T + blockdiag-ish LoRA) + bqkv
    W_eff = Wqkv.copy()
    W_eff[0:E] += (Bq_ @ Aq_).T
    W_eff[E:2 * E] += (Bk_ @ Ak_).T
    W_eff[2 * E:3 * E] += (Bv_ @ Av_).T
    # fp8 pre-scale: weights x32 (1/sqrt(D) and the scale unwind live in
    # the exp affine scale and the /32 on the msa weights)
    W_eff *= WSC
    bq_eff = bqkv[0:E] * WSC
    bk_eff = bqkv[E:2 * E] * WSC
    bv_raw = bqkv[2 * E:3 * E]          # unscaled V bias, folded into y_shard
    # msa: o @ Wmsa.T + o @ (Bo@Ao) = o @ M with M = Wmsa.T + Bo@Ao  [E(d), E(out)].
    # A2A payload carries 32*o; store 32*M in fp8 so the msa psum is 1024*msa,
    # and scale the residual (y + bv@M) by 1024 to match -- LayerNorm is
    # scale-invariant, so the final output is unchanged.
    M = (Wmsa.T + Bo_ @ Ao_)
    resid_bias = bv_raw @ M             # exact: (o+bv)@M == o@M + bv@M
    M_f8 = np.ascontiguousarray(M * WSC).astype(NP_F8)

    y_flat = y.reshape(T, E)
    yT_f8 = np.ascontiguousarray(y_flat.T).astype(NP_F8)

    use_gamma = not np.allclose(gamma, 1.0)
    use_beta = not np.allclose(beta, 0.0)
    nc = _get_nc(use_gamma, use_beta)

    in_maps = []
    for c in range(N_CORES):
        r0 = c * 128
        r1 = r0 + 128
        wq_c = np.ascontiguousarray(W_eff[0:E][r0:r1].T).astype(NP_F8)
        wk_c = np.ascontiguousarray(W_eff[E:2 * E][r0:r1].T).astype(NP_F8)
        wv_c = np.ascontiguousarray(W_eff[2 * E:3 * E][r0:r1].T).astype(NP_F8)
        tok = np.concatenate([
            np.arange(128 * c, 128 * c + 128),
            np.arange(1024 + 128 * c, 1024 + 128 * c + 128),
            np.arange(2048 + 128 * c, 2048 + 128 * c + 128),
            np.arange(3072 + 128 * c, 3072 + 128 * c + 128),
        ])
        m = {
            "yT": yT_f8,
            "wqT": wq_c,
            "wkT": wk_c,
            "wvT": wv_c,
            "bq": bq_eff[r0:r1].reshape(128, 1).copy(),
            "bk": bk_eff[r0:r1].reshape(128, 1).copy(),
            "msa_w": M_f8,
            "y_shard": np.ascontiguousarray(y_flat[tok] + resid_bias) * (WSC * WSC),
        }
        if use_gamma:
            m["gamma_b"] = np.broadcast_to(gamma, (128, E)).copy()
        if use_beta:
            m["beta_b"] = np.broadcast_to(beta, (128, E)).copy()
        in_maps.append(m)

    res = bass_utils.run_bass_kernel_spmd(nc, in_maps, core_ids=list(range(N_CORES)))

    out_full = np.empty((T, E), np.float32)
    for c in range(N_CORES):
        oc = res.results[c]["out"]
        out_full[128 * c:128 * c + 128] = oc[0:128]
        out_full[1024 + 128 * c:1024 + 128 * c + 128] = oc[128:256]
        out_full[2048 + 128 * c:2048 + 128 * c + 128] = oc[256:384]
        out_full[3072 + 128 * c:3072 + 128 * c + 128] = oc[384:512]
    return out_full.reshape(B, S, E)


# revision 46
# speedup vs baseline: 1.1218x; 1.1218x over previous
"""Trainium2 Bass kernel for fused LoRA-attention block (nn_Attention_18846316494887).

Reference computation:
  qkv = y @ Wqkv.T + bqkv (+ LoRA deltas y @ (B@A) per Q/K/V)  -> Q,K,V [B,H,S,D]
  attn = softmax(Q K^T / sqrt(D)); o = attn @ V -> [B,S,E]
  msa = o @ Wmsa.T + o @ (Bo@Ao); res = msa + y; out = LayerNorm(res)*gamma + beta

Sharding: tensor-parallel over heads (2 heads/core, 8 cores), AllToAll to
reshard head-dim -> token-dim before the output projection, token-parallel
msa + LayerNorm, host-side gather of per-core token shards.

Precision plan (error budget: attention path contributes only ~2.2% of the
LN'd output norm, so a few-% relative error there is invisible):
  - y, Wqkv (x32), V, exp(scores) all in fp8e4m3; f32 PSUM accumulation
  - Q/K projection matmuls in DoubleRow mode (2 fp8 k-subtiles per pass)
  - AV matmuls in DoubleRow mode over kt-pairs (halves the ex stream time)
  - the x32*x32 weight scaling and 1/sqrt(D) fold into the exp's free
    affine scale (exp(x * 1/8192)); V-scale folds into msa weights (/32)

Host-side prep (exact algebra, no approximation):
  - LoRA folded into Wqkv / Wmsa (y@W.T + y@(B@A) == y@(W.T + B@A))
  - V bias applied post-softmax on o (exact since attn rows sum to 1)
  - y pre-transposed to [E, T] for the QKV matmuls
"""
import functools
import numpy as np
import ml_dtypes

import concourse.mybir as mybir
import concourse.tile as tile
from concourse import bacc
from concourse import bass_utils
from concourse.bass import _add_dep_helper

# problem shapes (hardcoded per harness contract)
E = 1024
H = 16
D = 64
B = 2
S = 2048
T = B * S          # 4096 tokens
N_CORES = 8
EPS = 1e-6

BF16 = mybir.dt.bfloat16
F32 = mybir.dt.float32
F8 = mybir.dt.float8e4
NP_F8 = ml_dtypes.float8_e4m3
AF = mybir.ActivationFunctionType
ALU = mybir.AluOpType
DR = mybir.MatmulPerfMode.DoubleRow

# per-core worksizes
TOK = T // N_CORES          # 512 tokens per core for msa/LN
QC = 512                    # attention q-chunk
N_QC = S // QC              # 4 q-chunks per (b, head-pair)
N_KT = S // 128             # 16 k-tiles
N_KP = N_KT // 2            # 8 kt-pairs (DoubleRow AV granularity)
VW = 80                     # padded V row (64 d + 1 ones + pad to 16B mult)
WSC = 32.0                  # fp8 weight pre-scale
S_ACT = 1.0 / (WSC * WSC * 8.0)   # exp affine scale: /32^2 (w-scales) /sqrt(D)


def _build(use_gamma: bool, use_beta: bool):
    nc = bacc.Bacc("TRN2", target_bir_lowering=False, debug=False, num_devices=N_CORES)

    # ---- DRAM parameters -------------------------------------------------
    yT = nc.dram_tensor("yT", [E, T], F8, kind="ExternalInput")
    wqT = nc.dram_tensor("wqT", [E, 128], F8, kind="ExternalInput")
    wkT = nc.dram_tensor("wkT", [E, 128], F8, kind="ExternalInput")
    wvT = nc.dram_tensor("wvT", [E, 128], F8, kind="ExternalInput")
    bq = nc.dram_tensor("bq", [128, 1], F32, kind="ExternalInput")
    bk = nc.dram_tensor("bk", [128, 1], F32, kind="ExternalInput")
    bva = nc.dram_tensor("bva", [64, 1], F32, kind="ExternalInput")
    bvb = nc.dram_tensor("bvb", [64, 1], F32, kind="ExternalInput")
    msa_w = nc.dram_tensor("msa_w", [E, E], F8, kind="ExternalInput")
    y_shard = nc.dram_tensor("y_shard", [TOK, E], F32, kind="ExternalInput")
    if use_gamma:
        gamma_b = nc.dram_tensor("gamma_b", [128, E], F32, kind="ExternalInput")
    if use_beta:
        beta_b = nc.dram_tensor("beta_b", [128, E], F32, kind="ExternalInput")
    out = nc.dram_tensor("out", [TOK, E], F32, kind="ExternalOutput")

    # internal DRAM: A2A bounce buffers (shard k: (b, q-half) -> 128 tok/core)
    a2a_in = [nc.dram_tensor(f"a2a_in{k}", [N_CORES, 128, 128], F8) for k in range(4)]
    a2a_out = [nc.dram_tensor(f"a2a_out{k}", [N_CORES, 128, 128], F8) for k in range(4)]

    with tile.TileContext(nc) as tc:
        with (
            tc.tile_pool(name="const", bufs=1) as cpool,
            tc.tile_pool(name="yt", bufs=5) as ytp,
            tc.tile_pool(name="qk", bufs=1) as qkp,
            tc.tile_pool(name="exp", bufs=3) as expp,
            tc.tile_pool(name="stage", bufs=1) as stp,
            tc.tile_pool(name="fin", bufs=2) as finp,
            tc.tile_pool(name="a2asb", bufs=4) as a2ap,
            tc.tile_pool(name="ps_acc", bufs=2, space="PSUM") as ps_acc,
            tc.tile_pool(name="ps_sc", bufs=2, space="PSUM") as ps_sc,
            tc.tile_pool(name="ps_av", bufs=2, space="PSUM") as ps_av,
        ):
            # ---- constants -------------------------------------------------
            wqT_sb = cpool.tile([128, 8, 128], F8)
            wkT_sb = cpool.tile([128, 8, 128], F8)
            wvT_sb = cpool.tile([128, 8, 128], F8)
            # stripe the startup weight loads across both DMA queues so the
            # first yT tiles (behind them in queue order) land sooner
            nc.sync.dma_start(wqT_sb[:], wqT[:, :].rearrange("(a p) n -> p a n", p=128))
            nc.gpsimd.dma_start(wkT_sb[:], wkT[:, :].rearrange("(a p) n -> p a n", p=128))
            nc.sync.dma_start(wvT_sb[:], wvT[:, :].rearrange("(a p) n -> p a n", p=128))
            bq_sb = cpool.tile([128, 1], F32)
            bk_sb = cpool.tile([128, 1], F32)
            bva_sb = cpool.tile([64, 1], F32)
            bvb_sb = cpool.tile([64, 1], F32)
            nc.sync.dma_start(bq_sb[:], bq[:, :])
            nc.sync.dma_start(bk_sb[:], bk[:, :])
            nc.sync.dma_start(bva_sb[:], bva[:, :])
            nc.sync.dma_start(bvb_sb[:], bvb[:, :])
            # (msa weights / LN consts are DMA'd later, after the attention
            # loops are issued, so startup DMA bandwidth goes to yT tiles)
            msa_w_sb = cpool.tile([128, 8, E], F8)
            y_shard_sb = cpool.tile([128, 4, E], F32)
            if use_gamma:
                gamma_sb = cpool.tile([128, E], F32)
            if use_beta:
                beta_sb = cpool.tile([128, E], F32)

            # V tiles, padded: [k-part, b, head, kt, VW]; col 64 = ones
            v_sb = cpool.tile([128, B, 2, N_KT, VW], F8)
            nc.vector.memset(v_sb[:, :, :, :, 64:VW], 0.0)
            nc.vector.memset(v_sb[:, :, :, :, 64:65], 1.0)

            # Q^T/K^T: [d-part(2 heads), b, q]
            qT_sb = qkp.tile([128, B, S], BF16)
            kT_sb = qkp.tile([128, B, S], BF16)
            # o^T staging for A2A: [d-part, b, q] (fp8: carries 32*(o+bv))
            stage = stp.tile([128, B, S], F8)

            # ============== main per-batch pipeline ==============
            def make_qkv_steps(b):
                """QKV projection for batch b as a list of small closures so the
                PE work can be interleaved into the other batch's attention
                (fills the in-order PE stream's exp-wait slots)."""
                loads, qs, ks, vs = [], [], [], []
                for tc8 in range(4):
                    st8 = {}

                    def load(b=b, tc8=tc8, st8=st8):
                        yt = ytp.tile([128, 8, 512], F8, tag="yt")
                        st8["yt"] = yt
                        for et in range(8):
                            # b0 startup is DMA-latency-bound: stripe across
                            # the sync + gpsimd queues for 2x issue bandwidth.
                            # b1 loads run mid-kernel when sync is idle but
                            # gpsimd carries the finalize/A2A path: sync only.
                            eng = nc.gpsimd if (b == 0 and et % 2 == 1) else nc.sync
                            eng.dma_start(
                                yt[:, et, :], yT[128 * et:128 * (et + 1),
                                                 b * S + 512 * tc8: b * S + 512 * (tc8 + 1)])
                    loads.append(load)
                    qs.append([])
                    ks.append([])
                    vs.append([])

                    # Q/K: 4 DoubleRow matmuls (et-pairs), K=1024 contraction.
                    # DR forbids column tile_position offsets, so each mm is
                    # full-width [128, 2, 128] -> out [128, 512].
                    for eg in range(4):
                        def qstep(b=b, tc8=tc8, eg=eg, st8=st8):
                            if eg == 0:
                                st8["ps_q"] = ps_acc.tile([128, 512], F32, tag="acc", name="ps_q")
                            ps_q, yt = st8["ps_q"], st8["yt"]
                            st, sp = (eg == 0), (eg == 3)
                            nc.tensor.matmul(ps_q[:], wqT_sb[:, 2 * eg:2 * eg + 2, :],
                                             yt[:, 2 * eg:2 * eg + 2, :], start=st, stop=sp,
                                             perf_mode=DR)
                            if eg == 3:
                                nc.vector.tensor_scalar(
                                    qT_sb[:, b, 512 * tc8:512 * (tc8 + 1)], ps_q[:],
                                    bq_sb[:], None, ALU.add)
                        qs[tc8].append(qstep)

                    for eg in range(4):
                        def kstep(b=b, tc8=tc8, eg=eg, st8=st8):
                            if eg == 0:
                                st8["ps_k"] = ps_acc.tile([128, 512], F32, tag="acc", name="ps_k")
                            ps_k, yt = st8["ps_k"], st8["yt"]
                            st, sp = (eg == 0), (eg == 3)
                            nc.tensor.matmul(ps_k[:], wkT_sb[:, 2 * eg:2 * eg + 2, :],
                                             yt[:, 2 * eg:2 * eg + 2, :], start=st, stop=sp,
                                             perf_mode=DR)
                            if eg == 3:
                                nc.vector.tensor_scalar(
                                    kT_sb[:, b, 512 * tc8:512 * (tc8 + 1)], ps_k[:],
                                    bk_sb[:], None, ALU.add)
                        ks[tc8].append(kstep)

                    # V: [tok, vdim] layout, fp8 operands (no DoubleRow: the
                    # stationary operand changes every matmul)
                    for eg in range(4):
                        def vstep(b=b, tc8=tc8, eg=eg, st8=st8):
                            if eg == 0:
                                st8["ps_v"] = ps_acc.tile([128, 512], F32, tag="acc", name="ps_v")
                            ps_v, yt = st8["ps_v"], st8["yt"]
                            for et in (2 * eg, 2 * eg + 1):
                                st, sp = (et == 0), (et == 7)
                                for s4 in range(4):
                                    nc.tensor.matmul(ps_v[:, 128 * s4:128 * (s4 + 1)],
                                                     yt[:, et, 128 * s4:128 * (s4 + 1)],
                                                     wvT_sb[:, et, :], start=st, stop=sp)
                            if eg == 3:
                                for h in range(2):
                                    src = ps_v[:, :].rearrange(
                                        "p (s n) -> p s n", s=4)[:, :, 64 * h:64 * (h + 1)]
                                    nc.vector.tensor_copy(
                                        v_sb[:, b, h, 4 * tc8:4 * (tc8 + 1), 0:64], src)
                        vs[tc8].append(vstep)
                return loads, qs, ks, vs

            trigs = []  # collective trigger instrs, k-order

            def attention(b, bg, av_last=None, qcs=range(N_QC), pops=2):
                # software-pipelined ACROSS kt steps: qk/exp runs OV steps
                # ahead of av, so the ACT engine never drains at qc
                # boundaries; bg closures (other QKV work) fill PE wait slots.
                # AV runs per kt-PAIR in fp8 DoubleRow mode.
                if av_last is None:
                    av_last = []
                OV = 4
                states = {}

                def qk_exp(qc, kt):
                    stq = states[qc]
                    if kt % 2 == 0:
                        stq["exs"][kt // 2] = expp.tile([128, 2, 1024], F8, name="ex")
                    sc = ps_sc.tile([128, 1024], F32, tag="sc", name="sc")
                    nc.tensor.matmul(sc[:, 0:512],
                                     kT_sb[0:64, b, 128 * kt:128 * (kt + 1)],
                                     qT_sb[0:64, b, QC * qc:QC * (qc + 1)],
                                     start=True, stop=True, tile_position=(0, 0))
                    nc.tensor.matmul(sc[:, 512:1024],
                                     kT_sb[64:128, b, 128 * kt:128 * (kt + 1)],
                                     qT_sb[64:128, b, QC * qc:QC * (qc + 1)],
                                     start=True, stop=True, tile_position=(64, 0))
                    ex = stq["exs"][kt // 2]
                    nc.scalar.activation(ex[:, kt % 2, :], sc[:], AF.Exp, scale=S_ACT)

                def av_a(qc, kp):
                    stq = states[qc]
                    if kp == 0:
                        stq["av_a"] = ps_av.tile([128, 512], F32, tag="av", name="av_a")
                        stq["av_b"] = ps_av.tile([128, 512], F32, tag="av", name="av_b")
                    ex = stq["exs"][kp]
                    nc.tensor.matmul(stq["av_a"][0:65, :],
                                     v_sb[:, b, 0, 2 * kp:2 * kp + 2, 0:65],
                                     ex[:, :, 0:512],
                                     start=(kp == 0), stop=(kp == N_KP - 1), perf_mode=DR)

                def av_b(qc, kp):
                    stq = states[qc]
                    ex = stq["exs"][kp]
                    i2 = nc.tensor.matmul(stq["av_b"][0:65, :],
                                          v_sb[:, b, 1, 2 * kp:2 * kp + 2, 0:65],
                                          ex[:, :, 512:1024],
                                          start=(kp == 0), stop=(kp == N_KP - 1), perf_mode=DR)
                    if kp == N_KP - 1:
                        av_last.append(i2)

                def finalize(qc):
                    av_a, av_b = states[qc]["av_a"], states[qc]["av_b"]
                    # drain AV psum to SBUF fast (releases psum for next q-chunk)
                    af = finp.tile([128, 1024], F32, tag="af", name="af")
                    nc.vector.tensor_copy(af[0:65, 0:512], av_a[0:65, :])
                    nc.vector.tensor_copy(af[0:65, 512:1024], av_b[0:65, :])
                    # denominator row -> partition 0 (DMA shifts partitions;
                    # reciprocal_approx_fast corrupts on non-zero base
                    # partitions, so the recip must run at partition 0),
                    # then gpsimd broadcast to all lanes
                    rc = finp.tile([128, 1024], F32, tag="rc", name="rc")
                    nc.gpsimd.dma_start(rc[0:1, :], af[64:65, :])
                    rc2 = finp.tile([128, 1024], F32, tag="rc2", name="rc2")
                    nc.vector.reciprocal_approx_fast(rc2[0:1, :], rc[0:1, :])
                    rb = finp.tile([128, 1024], F32, tag="rb", name="rb")
                    nc.gpsimd.partition_broadcast(rb[:, :], rc2[0:1, :])
                    # o^T = o_raw^T * recip + bv; all on partitions 0..63, then
                    # head B is partition-shifted into the stage via DMA.
                    # (fp8 tiles are write-only for the DVE: mult lands in an
                    # f32 scratch, the bias-add writes the fp8 copy once)
                    osc = stage[:, b, QC * qc:QC * (qc + 1)]
                    om = finp.tile([64, 1024], F32, tag="om", name="om")
                    nc.vector.tensor_tensor(om[:, 0:512], af[0:64, 0:512], rb[0:64, 0:512], ALU.mult)
                    nc.vector.tensor_scalar(om[:, 0:512], om[:, 0:512], bva_sb[:], None, ALU.add)
                    nc.vector.tensor_copy(osc[0:64, :], om[:, 0:512])
                    tb = finp.tile([64, 512], F8, tag="tb", name="tb")
                    nc.vector.tensor_tensor(om[:, 512:1024], af[0:64, 512:1024], rb[0:64, 512:1024], ALU.mult)
                    nc.vector.tensor_scalar(om[:, 512:1024], om[:, 512:1024], bvb_sb[:], None, ALU.add)
                    nc.vector.tensor_copy(tb[:], om[:, 512:1024])
                    nc.gpsimd.dma_start(osc[64:128, :], tb[:])
                    # A2A per q-half: upload each qc's blocks as soon as
                    # staged; issue the collective after the odd qc
                    hf = qc // 2
                    k = 2 * b + hf
                    half = a2a_in[k].ap().rearrange("j p n -> p j n")
                    if qc % 2 == 0:
                        nc.gpsimd.dma_start(
                            half[:, 0:4, :],
                            stage[:, b, 1024 * hf:1024 * hf + 512].rearrange(
                                "p (j n) -> p j n", j=4))
                    else:
                        nc.gpsimd.dma_start(
                            half[:, 4:8, :],
                            stage[:, b, 1024 * hf + 512:1024 * (hf + 1)].rearrange(
                                "p (j n) -> p j n", j=4))
                        trigs.append(nc.gpsimd.collective_compute(
                            "AllToAll", ALU.bypass,
                            replica_groups=[list(range(N_CORES))],
                            ins=[a2a_in[k].ap().opt()],
                            outs=[a2a_out[k].ap().opt()],
                        ))

                seq = [(qc, kt) for qc in qcs for kt in range(N_KT)]
                for i, (qc, kt) in enumerate(seq):
                    states.setdefault(qc, {"exs": [None] * N_KP})
                    qk_exp(qc, kt)
                    for _ in range(pops):
                        if bg:
                            bg.pop(0)()
                    j = i - OV
                    if j >= 0 and seq[j][1] % 2 == 1:
                        jqc, jkt = seq[j]
                        av_a(jqc, jkt // 2)
                        av_b(jqc, jkt // 2)
                        if jkt == N_KT - 1:
                            finalize(jqc)
                for j in range(max(0, len(seq) - OV), len(seq)):
                    if seq[j][1] % 2 == 1:
                        jqc, jkt = seq[j]
                        av_a(jqc, jkt // 2)
                        av_b(jqc, jkt // 2)
                        if jkt == N_KT - 1:
                            finalize(jqc)
                return av_last

            # drive: emit only chunk 0 of b0's QKV up front, then start
            # attention qc0 with chunks 1-3 interleaved as background steps
            # (order [K,V] per chunk matches the kt windows that consume them);
            # b1's QKV interleaves into b0's qc1-3.
            l0, q0, k0, v0 = make_qkv_steps(0)
            for step in l0:
                step()
            for s in k0[0]:
                s()
            for s in q0[0]:
                s()
            for s in v0[0]:
                s()
            l1, q1, k1, v1 = make_qkv_steps(1)
            # bgA feeds b0 qc0: kT chunk c needed by step 4c, v chunk c by the
            # av of its kt-pairs, q0[1] before qc1 starts; 28 closures, 32 slots
            bgA = []
            for tc8 in (1, 2, 3):
                bgA.extend(k0[tc8])
                bgA.extend(v0[tc8])
            bgA.extend(q0[1])
            attention(0, bgA, qcs=[0])
            while bgA:
                bgA.pop(0)()
            # qc1: rest of b0's Q + b1's loads and kT (25 closures, 32 slots)
            bgB = list(q0[2]) + list(q0[3]) + list(l1)
            for tc8 in range(4):
                bgB.extend(k1[tc8])
            attention(0, bgB, qcs=[1])
            while bgB:
                bgB.pop(0)()
            # qc2+qc3: b1's Q and V (32 closures, 32 slots)
            bgC = []
            for tc8 in range(4):
                bgC.extend(q1[tc8])
                bgC.extend(v1[tc8])
            attention(0, bgC, qcs=[2, 3], pops=1)
            while bgC:
                bgC.pop(0)()
            av_anchors = attention(1, [])

            # deferred bulk const loads (issued after attention DMAs in queue order)
            nc.sync.dma_start(msa_w_sb[:], msa_w[:, :].rearrange("(a p) n -> p a n", p=128))
            nc.sync.dma_start(y_shard_sb[:], y_shard[:, :].rearrange("(a p) n -> p a n", p=128))
            if use_gamma:
                nc.sync.dma_start(gamma_sb[:], gamma_b[:, :])
            if use_beta:
                nc.sync.dma_start(beta_sb[:], beta_b[:, :])

            # ============== msa + residual + LayerNorm per shard ==============
            res_sb = stp.tile([128, 4, E], F32)
            for k in range(4):
                lhs = a2ap.tile([128, 8, 128], F8, tag="lhs")
                nc.sync.dma_start(lhs[:], a2a_out[k].ap().rearrange("j p n -> p j n"))
                # i-major so consecutive matmuls share lhs weights (LDW dedup);
                # both e-halves accumulate concurrently in two psum tiles.
                # fp8 DoubleRow: i-pairs, contraction 1024 in 4 passes.
                ps_m0 = ps_acc.tile([128, 512], F32, tag="acc", name="ps_m0")
                ps_m1 = ps_acc.tile([128, 512], F32, tag="acc", name="ps_m1")
                for i in range(4):
                    for ec, ps_m in ((0, ps_m0), (1, ps_m1)):
                        mi = nc.tensor.matmul(ps_m[:], lhs[:, 2 * i:2 * i + 2, :],
                                              msa_w_sb[:, 2 * i:2 * i + 2,
                                                       512 * ec:512 * (ec + 1)],
                                              start=(i == 0), stop=(i == 3),
                                              perf_mode=DR)
                        if ec == 0 and i == 0:
                            # keep msa out of the PE stream until b1 attention
                            # has progressed past qc k+1 (the A2A data won't be
                            # there earlier; an early msa blocks the in-order PE)
                            _add_dep_helper(
                                mi.ins, av_anchors[min(k + 1, 3)].ins, sync=False,
                                reason="msa gated behind b1 attention progress")
                for ec, ps_m in ((0, ps_m0), (1, ps_m1)):
                    # residual add, on DVE (no ACT table switch)
                    rhalf = res_sb[:, k, 512 * ec:512 * (ec + 1)]
                    ri = nc.vector.tensor_tensor(
                        rhalf, ps_m[:],
                        y_shard_sb[:, k, 512 * ec:512 * (ec + 1)], ALU.add)
                    if ec == 0:
                        # keep this shard's LN work behind the (k+1)-th
                        # collective TRIGGER on the DVE queue: the trigger path
                        # of the last q-chunk must not queue behind LN ops.
                        # NOTE: sync=False on purpose -- a sync=True semaphore
                        # edge here deadlocks the device (cross-engine cycle)
                        _add_dep_helper(
                            ri.ins, trigs[min(k + 1, 3)].ins, sync=False,
                            reason="LN deprioritized behind collective trigger")
                # fused mean/var via bn_stats halves + one aggregate
                stats = finp.tile([128, 2, 6], F32, tag="stats")
                nc.vector.bn_stats(stats[:, 0, :], res_sb[:, k, 0:512])
                nc.vector.bn_stats(stats[:, 1, :], res_sb[:, k, 512:1024])
                mu = cpool.tile([128, 4], F32, name=f"mu{k}")
                nc.vector.bn_aggr(mu[:, 0:2], stats[:])
                # rstd = sqrt(1/(var+eps)); 51-ULP reciprocal is plenty here
                nc.vector.tensor_scalar(mu[:, 1:2], mu[:, 1:2], EPS, None, ALU.add)
                nc.vector.reciprocal_approx_fast(mu[:, 2:3], mu[:, 1:2])
                nc.scalar.activation(mu[:, 3:4], mu[:, 2:3], AF.Sqrt)
                nc.vector.tensor_scalar(mu[:, 0:1], mu[:, 0:1], -1.0, None, ALU.mult)
                # o1 = (res - mu) * rstd, fused on DVE
                o1 = finp.tile([128, E], F32, tag="o1")
                nc.vector.tensor_scalar(o1[:], res_sb[:, k, :], mu[:, 0:1],
                                        mu[:, 3:4], ALU.add, ALU.mult)
                if use_gamma:
                    nc.vector.tensor_tensor(o1[:], o1[:], gamma_sb[:], ALU.mult)
                if use_beta:
                    nc.vector.tensor_tensor(o1[:], o1[:], beta_sb[:], ALU.add)
                nc.sync.dma_start(out[128 * k:128 * (k + 1), :], o1[:])

    nc.compile()
    return nc


@functools.lru_cache(maxsize=4)
def _get_nc(use_gamma: bool, use_beta: bool):
    return _build(use_gamma, use_beta)


def kernel(**inputs) -> np.ndarray:
    y = np.asarray(inputs["y"], np.float32)
    Wqkv = np.asarray(inputs["Wqkv"], np.float32)
    bqkv = np.asarray(inputs["bqkv"], np.float32)
    Wmsa = np.asarray(inputs["Wmsa"], np.float32)
    Bq_, Aq_ = np.asarray(inputs["Bq"], np.float32), np.asarray(inputs["Aq"], np.float32)
    Bk_, Ak_ = np.asarray(inputs["Bk"], np.float32), np.asarray(inputs["Ak"], np.float32)
    Bv_, Av_ = np.asarray(inputs["Bv"], np.float32), np.asarray(inputs["Av"], np.float32)
    Bo_, Ao_ = np.asarray(inputs["Bo"], np.float32), np.asarray(inputs["Ao"], np.float32)
    gamma = np.asarray(inputs["gamma"], np.float32)
    beta = np.asarray(inputs["beta"], np.float32)

    # effective weights: qkv = y @ (Wqkv.T + blockdiag-ish LoRA) + bqkv
    # y @ W.T: W rows are output dims. LoRA adds y @ (B@A): effective W += (B@A).T
    W_eff = Wqkv.copy()
    W_eff[0:E] += (Bq_ @ Aq_).T
    W_eff[E:2 * E] += (Bk_ @ Ak_).T
    W_eff[2 * E:3 * E] += (Bv_ @ Av_).T
    # fp8 pre-scale: weights x32 (1/sqrt(D) and the scale unwind live in
    # the exp affine scale and the /32 on the msa weights)
    W_eff *= WSC
    bq_eff = bqkv[0:E] * WSC
    bk_eff = bqkv[E:2 * E] * WSC
    bv_eff = bqkv[2 * E:3 * E] * WSC
    # msa: o @ Wmsa.T + o @ (Bo@Ao) = o @ M with M = Wmsa.T + Bo@Ao  [E(d), E(out)].
    # stage carries 32*(o+bv); store 32*M in fp8 (good dynamic range) so the
    # msa psum is 1024*msa, and scale the residual y by 1024 to match --
    # LayerNorm is scale-invariant, so the final output is unchanged.
    M = (Wmsa.T + Bo_ @ Ao_) * WSC

    y_flat = y.reshape(T, E)
    yT_f8 = np.ascontiguousarray(y_flat.T).astype(NP_F8)
    M_f8 = np.ascontiguousarray(M).astype(NP_F8)

    use_gamma = not np.allclose(gamma, 1.0)
    use_beta = not np.allclose(beta, 0.0)
    nc = _get_nc(use_gamma, use_beta)

    in_maps = []
    for c in range(N_CORES):
        r0 = c * 128
        r1 = r0 + 128
        wq_c = np.ascontiguousarray(W_eff[0:E][r0:r1].T).astype(NP_F8)
        wk_c = np.ascontiguousarray(W_eff[E:2 * E][r0:r1].T).astype(NP_F8)
        wv_c = np.ascontiguousarray(W_eff[2 * E:3 * E][r0:r1].T).astype(NP_F8)
        tok = np.concatenate([
            np.arange(128 * c, 128 * c + 128),
            np.arange(1024 + 128 * c, 1024 + 128 * c + 128),
            np.arange(2048 + 128 * c, 2048 + 128 * c + 128),
            np.arange(3072 + 128 * c, 3072 + 128 * c + 128),
        ])
        m = {
            "yT": yT_f8,
            "wqT": wq_c,
            "wkT": wk_c,
            "wvT": wv_c,
            "bq": bq_eff[r0:r1].reshape(128, 1).copy(),
            "bk": bk_eff[r0:r1].reshape(128, 1).copy(),
            "bva": bv_eff[r0:r0 + 64].reshape(64, 1).copy(),
            "bvb": bv_eff[r0 + 64:r1].reshape(64, 1).copy(),
            "msa_w": M_f8,
            "y_shard": np.ascontiguousarray(y_flat[tok]) * (WSC * WSC),
        }
        if use_gamma:
            m["gamma_b"] = np.broadcast_to(gamma, (128, E)).copy()
        if use_beta:
            m["beta_b"] = np.broadcast_to(beta, (128, E)).copy()
        in_maps.append(m)

    res = bass_utils.run_bass_kernel_spmd(nc, in_maps, core_ids=list(range(N_CORES)))

    out_full = np.empty((T, E), np.float32)
    for c in range(N_CORES):
        oc = res.results[c]["out"]
        out_full[128 * c:128 * c + 128] = oc[0:128]
        out_full[1024 + 128 * c:1024 + 128 * c + 128] = oc[128:256]
        out_full[2048 + 128 * c:2048 + 128 * c + 128] = oc[256:384]
        out_full[3072 + 128 * c:3072 + 128 * c + 128] = oc[384:512]
    return out_full.reshape(B, S, E)

